# revision 3
# baseline (speedup 1.0000x reference)
"""Trainium2 Bass kernel for the kNN pairwise-ranking loss.

Math: with y = (knn_tgts == tgts), the masked pairwise BCE-with-logits loss
over differing-label pairs (j > i) collapses to

    loss = sum_b sum_{n in neg_b} sum_{p in pos_b} softplus(s_n - s_p) / cnt
    cnt  = sum_b |pos_b| * |neg_b|

because for a (pos, neg) pair the per-pair term is softplus(s_neg - s_pos)
regardless of orientation, and b2 cancels in score differences.

Host side: per batch row, permute keys so positives come first, then
negatives, then masked-out entries.  Two additive pad vectors (+PAD on
non-positives, -PAD on non-negatives) push padded scores far out so their
softplus contribution underflows to exactly ln(1) = 0; the device then just
sums a dense [neg-chunks x Jmax] softplus block with no indicator masking.

Device (SPMD over 8 cores, 4 batch rows each):
  phase A (per row): h = relu(W1 @ keys^T + b1) via PE (bf16 in, f32 psum),
                     s_row [1,Jmax] and s_col chunks [128,1] via small PE
                     matmuls off the same bf16 h tiles
  phase B (per row): s_pos broadcast via ones-matmul (f32), then per
                     128-neg-chunk ACT exp followed by ln(x+1) — softplus —
                     with accum_out giving per-partition row sums directly.
Host gathers [128, rows*chunks] partial sums, reduces, divides by cnt.
"""

import numpy as np

B, K, D, H = 32, 1024, 1024, 100
N_CORES = 8
BPC = B // N_CORES  # batch rows per core
PAD = 60.0
USE_BF16 = True

_cache = {}


def _build_program(Jmax, c0, use_bf16):
    import concourse.bacc as bacc
    import concourse.mybir as mybir
    import concourse.tile as tile

    f32 = mybir.dt.float32
    kdt = mybir.dt.bfloat16 if use_bf16 else f32
    nch = 8 - c0

    nc = bacc.Bacc(
        "TRN2",
        target_bir_lowering=False,
        debug=False,
        enable_asserts=False,
        num_devices=N_CORES,
    )

    keys_d = nc.dram_tensor("keys_t", [BPC, D, K], kdt, kind="ExternalInput").ap()
    w1t_d = nc.dram_tensor("w1t", [D, H], kdt, kind="ExternalInput").ap()
    w2_d = nc.dram_tensor("w2c", [H, 1], kdt, kind="ExternalInput").ap()
    b1_d = nc.dram_tensor("b1c", [H, 1], f32, kind="ExternalInput").ap()
    pospad_d = nc.dram_tensor("pospad", [BPC, Jmax], f32, kind="ExternalInput").ap()
    negcol_d = nc.dram_tensor("negcol", [BPC, 128, 8], f32, kind="ExternalInput").ap()
    out_d = nc.dram_tensor("acc_out", [128, BPC * nch], f32, kind="ExternalOutput").ap()

    with tile.TileContext(nc) as tc:
        with (
            tc.tile_pool(name="const", bufs=1) as cpool,
            tc.tile_pool(name="keys", bufs=6) as kpool,
            tc.tile_pool(name="h", bufs=3) as hpool,
            tc.tile_pool(name="svec", bufs=2) as spool,
            tc.tile_pool(name="tsp", bufs=2) as tpool,
            tc.tile_pool(name="hp", bufs=2, space="PSUM") as hp_pool,
            tc.tile_pool(name="sp", bufs=2, space="PSUM") as sp_pool,
            tc.tile_pool(name="scp", bufs=1, space="PSUM") as scp_pool,
            tc.tile_pool(name="pbp", bufs=2, space="PSUM") as pb_pool,
        ):
            # ---- constants ----
            w1t_sb = cpool.tile([128, 8 * H], kdt, tag="w1t")
            for dc in range(8):
                nc.sync.dma_start(
                    w1t_sb[:, dc * H : (dc + 1) * H],
                    w1t_d[dc * 128 : (dc + 1) * 128, :],
                )
            w2_sb = cpool.tile([H, 1], kdt, tag="w2")
            nc.sync.dma_start(w2_sb[:], w2_d[:])
            b1_sb = cpool.tile([H, 1], f32, tag="b1")
            nc.sync.dma_start(b1_sb[:], b1_d[:])
            ones_sb = cpool.tile([1, 128], f32, tag="ones")
            nc.vector.memset(ones_sb[:], 1.0)
            acc_sb = cpool.tile([128, BPC * nch], f32, tag="acc")

            for b in range(BPC):
                # ---- phase A: MLP scores ----
                hp0 = hp_pool.tile([H, 512], f32, tag="hp")
                hp1 = hp_pool.tile([H, 512], f32, tag="hp")
                for dc in range(8):
                    kt = kpool.tile([128, K], kdt, tag="keys")
                    nc.sync.dma_start(kt[:], keys_d[b, dc * 128 : (dc + 1) * 128, :])
                    w_sl = w1t_sb[:, dc * H : (dc + 1) * H]
                    nc.tensor.matmul(
                        hp0[:], lhsT=w_sl, rhs=kt[:, 0:512],
                        start=(dc == 0), stop=(dc == 7),
                    )
                    nc.tensor.matmul(
                        hp1[:], lhsT=w_sl, rhs=kt[:, 512:1024],
                        start=(dc == 0), stop=(dc == 7),
                    )
                # relu(h + b1): PSUM -> SBUF on DVE (cast to kdt for matmul-2)
                h0 = hpool.tile([H, 512], kdt, tag="h")
                h1 = hpool.tile([H, 512], kdt, tag="h")
                nc.vector.tensor_scalar(
                    h0[:], hp0[:], b1_sb[:], 0.0,
                    op0=mybir.AluOpType.add, op1=mybir.AluOpType.max,
                )
                nc.vector.tensor_scalar(
                    h1[:], hp1[:], b1_sb[:], 0.0,
                    op0=mybir.AluOpType.add, op1=mybir.AluOpType.max,
                )
                # s_row for the positive (free) side: only first Jmax needed
                sr_ps = sp_pool.tile([1, 512], f32, tag="sr")
                nc.tensor.matmul(
                    sr_ps[:], lhsT=w2_sb[:], rhs=h0[:], start=True, stop=True
                )
                # s_col chunks for the negative (partition) side
                sc_ps = scp_pool.tile([128, nch], f32, tag="sc")
                for c in range(c0, 8):
                    hsrc = h0 if c < 4 else h1
                    off = (c * 128) % 512
                    nc.tensor.matmul(
                        sc_ps[:, c - c0 : c - c0 + 1],
                        lhsT=hsrc[:, off : off + 128],
                        rhs=w2_sb[:],
                        start=True, stop=True,
                    )
                # pads
                pp_sb = spool.tile([1, Jmax], f32, tag="pp")
                nc.sync.dma_start(pp_sb[:], pospad_d[b : b + 1, :])
                ngc_sb = spool.tile([128, 8], f32, tag="ngc")
                nc.sync.dma_start(ngc_sb[:], negcol_d[b, :, :])
                spos_sb = spool.tile([1, Jmax], f32, tag="spos")
                nc.vector.tensor_add(spos_sb[:], sr_ps[0:1, 0:Jmax], pp_sb[:])
                sneg_sb = spool.tile([128, nch], f32, tag="sneg")
                nc.vector.tensor_add(sneg_sb[:], sc_ps[:], ngc_sb[:, c0:8])
                # broadcast s_pos across partitions (f32 matmul, small)
                pb_ps = pb_pool.tile([128, Jmax], f32, tag="pb")
                nc.tensor.matmul(
                    pb_ps[:], lhsT=ones_sb[:], rhs=spos_sb[:], start=True, stop=True
                )
                sbc_sb = spool.tile([128, Jmax], f32, tag="sbc")
                nc.vector.tensor_copy(sbc_sb[:], pb_ps[:])
                # ---- phase B: softplus(s_neg_i - s_pos_j), accumulate over j ----
                # softplus(x) = ln(exp(x) + 1); exp and ln live in one ACT
                # table set (natural_log_exp_and_others) so no table switches.
                for c in range(nch):
                    et = tpool.tile([128, Jmax], f32, tag="tsp")
                    nc.scalar.activation(
                        et[:], sbc_sb[:],
                        mybir.ActivationFunctionType.Exp,
                        bias=sneg_sb[:, c : c + 1], scale=-1.0,
                    )
                    tt = tpool.tile([128, Jmax], f32, tag="tsp2")
                    nc.scalar.activation(
                        tt[:], et[:],
                        mybir.ActivationFunctionType.Ln,
                        bias=1.0, scale=1.0,
                        accum_out=acc_sb[:, b * nch + c : b * nch + c + 1],
                    )

            nc.sync.dma_start(out_d[:], acc_sb[:])

    nc.compile()
    return nc


def kernel(keys, tgts, knn_tgts, mask, W1, b1, W2, b2, _profile=False):
    import ml_dtypes

    from concourse.bass_utils import run_bass_kernel_spmd

    keys = np.asarray(keys, dtype=np.float32)
    tgts = np.asarray(tgts)
    knn_tgts = np.asarray(knn_tgts)
    mask = np.asarray(mask).astype(bool)
    W1 = np.asarray(W1, dtype=np.float32)
    b1 = np.asarray(b1, dtype=np.float32)
    W2 = np.asarray(W2, dtype=np.float32)

    # ---- host-side label/permutation prep ----
    y = knn_tgts == tgts[:, None]
    pos = y & mask
    neg = (~y) & mask
    P = pos.sum(axis=1)
    N_ = neg.sum(axis=1)
    cnt = float((P.astype(np.int64) * N_.astype(np.int64)).sum())

    # stable order: positives, negatives, masked-out
    rank = np.where(pos, 0, np.where(neg, 1, 2)).astype(np.int8)
    order = np.argsort(rank, axis=1, kind="stable")  # [B, K]

    Pmax = int(P.max())
    Pmin = int(P.min())
    Jmax = min(512, ((Pmax + 7) // 8) * 8)
    assert Pmax <= 512, f"positive count {Pmax} > 512 unsupported"
    if Jmax < 64:
        Jmax = 64
    c0 = Pmin // 128
    nch = 8 - c0

    # permuted, transposed keys: [B, D, K]
    keys_perm = np.take_along_axis(keys, order[:, :, None], axis=1)  # [B, K, D]
    keys_t = np.ascontiguousarray(keys_perm.transpose(0, 2, 1))
    kdt = ml_dtypes.bfloat16 if USE_BF16 else np.float32
    keys_t = keys_t.astype(kdt)

    # pads in permuted coordinates
    kidx = np.arange(K)[None, :]
    pospad = np.where(kidx < P[:, None], 0.0, PAD).astype(np.float32)  # [B, K]
    negpad = np.where(
        (kidx >= P[:, None]) & (kidx < (P + N_)[:, None]), 0.0, -PAD
    ).astype(np.float32)
    pospad = np.ascontiguousarray(pospad[:, :Jmax])
    negcol = np.ascontiguousarray(
        negpad.reshape(B, 8, 128).transpose(0, 2, 1)
    )  # [B, 128, 8]

    w1t = np.ascontiguousarray(W1.T).astype(kdt)  # [D, H]
    w2c = np.ascontiguousarray(W2.reshape(1, H).T).astype(kdt)  # [H, 1]
    b1c = np.ascontiguousarray(b1.reshape(H, 1))

    key = (Jmax, c0, USE_BF16)
    if key not in _cache:
        _cache[key] = _build_program(Jmax, c0, USE_BF16)
    nc = _cache[key]

    in_maps = []
    for c in range(N_CORES):
        sl = slice(c * BPC, (c + 1) * BPC)
        in_maps.append(
            {
                "keys_t": keys_t[sl],
                "w1t": w1t,
                "w2c": w2c,
                "b1c": b1c,
                "pospad": pospad[sl],
                "negcol": negcol[sl],
            }
        )

    res = run_bass_kernel_spmd(
        nc, in_maps, list(range(N_CORES)), trace=bool(_profile)
    )
    total = 0.0
    for r in res.results:
        total += float(r["acc_out"].astype(np.float64).sum())
    if _profile:
        print(f"HW exec time: {res.exec_time_ns} ns")
        globals()["_last_results"] = res
    loss = np.float64(total) / np.float64(cnt)
    return np.array(loss, dtype=np.float32)


# revision 4
# speedup vs baseline: 1.5349x; 1.5349x over previous
"""Trainium2 Bass kernel for the kNN pairwise-ranking loss.

Math: with y = (knn_tgts == tgts), the masked pairwise BCE-with-logits loss
over differing-label pairs (j > i) collapses to

    loss = sum_b sum_{n in neg_b} sum_{p in pos_b} softplus(s_n - s_p) / cnt
    cnt  = sum_b |pos_b| * |neg_b|

because for a (pos, neg) pair the per-pair term is softplus(s_neg - s_pos)
regardless of orientation, and b2 cancels in score differences.

Host side: per batch row, permute keys so positives come first, then
negatives, then masked-out entries.  Additive pad vectors (+PAD on
non-positives, -PAD on non-negatives) push padded scores far out so their
softplus contribution underflows to exactly ln(1) = 0; the device then just
sums a dense [pos-chunks x neg-width] softplus block with no masking.

Device (SPMD over 8 cores, 4 batch rows each):
  phase A (per row): h = relu(W1 @ keys^T + b1) via PE (bf16 in, f32 psum),
                     s_row [1,K] and s_col chunks [128,1] via small PE
                     matmuls off the same bf16 h tiles
  phase B (per row): positives on partitions (few chunks), negatives along
                     the free dim; softplus(s_neg_i - s_pos_j) computed as
                     exp then ln(x+1) on ACT (both funcs in ONE table set,
                     enforced by the get_activation_tables patch below),
                     with accum_out giving per-partition sums for free.
Host gathers [128, rows*chunks] partial sums, reduces, divides by cnt.
"""

import numpy as np

B, K, D, H = 32, 1024, 1024, 100
N_CORES = 8
BPC = B // N_CORES  # batch rows per core
PAD = 60.0
USE_BF16 = True

_cache = {}
_act_patched = False


def _patch_act_tables():
    """Make Exp/Ln resolve to the single combined ACT table set.

    bass_rust's act-table-load inserter picks, per activation, some set
    containing the needed function; Exp and Ln naturally resolve to two
    different sets, causing a ~1.3us ACT_TABLE_LOAD on every exp<->ln
    transition.  natural_log_exp_and_others contains both, so restricting
    the registry to it for exp/ln yields exactly one load per kernel.
    """
    global _act_patched
    if _act_patched:
        return
    import concourse.bacc as bacc
    import concourse.hw_specs as hw_specs
    import concourse.mybir as mybir

    orig = hw_specs.get_activation_tables
    combined = "natural_log_exp_and_others"

    def patched(arch):
        tabs = orig(arch)
        out = {}
        for name, funcs in tabs.items():
            f = set(funcs)
            if name != combined and combined in tabs:
                f.discard(mybir.ActivationFunctionType.Exp)
                f.discard(mybir.ActivationFunctionType.Ln)
            out[name] = f
        return out

    hw_specs.get_activation_tables = patched
    bacc.get_activation_tables = patched
    _act_patched = True


def _build_program(Jmax, nst, use_bf16):
    import concourse.bacc as bacc
    import concourse.mybir as mybir
    import concourse.tile as tile

    _patch_act_tables()

    f32 = mybir.dt.float32
    kdt = mybir.dt.bfloat16 if use_bf16 else f32
    npch = (Jmax + 127) // 128  # positive-side partition chunks
    nw = K - nst  # negative-side free width

    nc = bacc.Bacc(
        "TRN2",
        target_bir_lowering=False,
        debug=False,
        enable_asserts=False,
        num_devices=N_CORES,
    )

    keys_d = nc.dram_tensor("keys_t", [BPC, D, K], kdt, kind="ExternalInput").ap()
    w1t_d = nc.dram_tensor("w1t", [D, H], kdt, kind="ExternalInput").ap()
    w2_d = nc.dram_tensor("w2c", [H, 1], kdt, kind="ExternalInput").ap()
    b1_d = nc.dram_tensor("b1c", [H, 1], f32, kind="ExternalInput").ap()
    ppcol_d = nc.dram_tensor("ppcol", [BPC, 128, npch], f32, kind="ExternalInput").ap()
    negrow_d = nc.dram_tensor("negrow", [BPC, nw], f32, kind="ExternalInput").ap()
    out_d = nc.dram_tensor(
        "acc_out", [128, BPC * npch], f32, kind="ExternalOutput"
    ).ap()

    with tile.TileContext(nc) as tc:
        with (
            tc.tile_pool(name="const", bufs=1) as cpool,
            tc.tile_pool(name="keys", bufs=8) as kpool,
            tc.tile_pool(name="h", bufs=3) as hpool,
            tc.tile_pool(name="svec", bufs=2) as spool,
            tc.tile_pool(name="tsp", bufs=3) as tpool,
            tc.tile_pool(name="hp", bufs=2, space="PSUM") as hp_pool,
            tc.tile_pool(name="sp", bufs=1, space="PSUM") as sp_pool,
            tc.tile_pool(name="scp", bufs=1, space="PSUM") as scp_pool,
            tc.tile_pool(name="pbp", bufs=1, space="PSUM") as pb_pool,
        ):
            # ---- constants ----
            w1t_sb = cpool.tile([128, 8 * H], kdt, tag="w1t")
            for dc in range(8):
                nc.sync.dma_start(
                    w1t_sb[:, dc * H : (dc + 1) * H],
                    w1t_d[dc * 128 : (dc + 1) * 128, :],
                )
            w2_sb = cpool.tile([H, 1], kdt, tag="w2")
            nc.sync.dma_start(w2_sb[:], w2_d[:])
            b1_sb = cpool.tile([H, 1], f32, tag="b1")
            nc.sync.dma_start(b1_sb[:], b1_d[:])
            ones_sb = cpool.tile([1, 128], f32, tag="ones")
            nc.vector.memset(ones_sb[:], 1.0)
            acc_sb = cpool.tile([128, BPC * npch], f32, tag="acc")

            for b in range(BPC):
                # ---- phase A: MLP scores ----
                hp0 = hp_pool.tile([H, 512], f32, tag="hp")
                hp1 = hp_pool.tile([H, 512], f32, tag="hp")
                for dc in range(8):
                    kt = kpool.tile([128, K], kdt, tag="keys")
                    nc.sync.dma_start(kt[:], keys_d[b, dc * 128 : (dc + 1) * 128, :])
                    w_sl = w1t_sb[:, dc * H : (dc + 1) * H]
                    nc.tensor.matmul(
                        hp0[:], lhsT=w_sl, rhs=kt[:, 0:512],
                        start=(dc == 0), stop=(dc == 7),
                    )
                    nc.tensor.matmul(
                        hp1[:], lhsT=w_sl, rhs=kt[:, 512:1024],
                        start=(dc == 0), stop=(dc == 7),
                    )
                # relu(h + b1): PSUM -> SBUF on DVE (cast to kdt for matmul-2)
                h0 = hpool.tile([H, 512], kdt, tag="h")
                h1 = hpool.tile([H, 512], kdt, tag="h")
                nc.vector.tensor_scalar(
                    h0[:], hp0[:], b1_sb[:], 0.0,
                    op0=mybir.AluOpType.add, op1=mybir.AluOpType.max,
                )
                nc.vector.tensor_scalar(
                    h1[:], hp1[:], b1_sb[:], 0.0,
                    op0=mybir.AluOpType.add, op1=mybir.AluOpType.max,
                )
                # s_row over the full row (negative / free side uses a slice)
                sr_ps = sp_pool.tile([1, 1024], f32, tag="sr")
                nc.tensor.matmul(
                    sr_ps[0:1, 0:512], lhsT=w2_sb[:], rhs=h0[:],
                    start=True, stop=True,
                )
                nc.tensor.matmul(
                    sr_ps[0:1, 512:1024], lhsT=w2_sb[:], rhs=h1[:],
                    start=True, stop=True,
                )
                # s_col chunks for the positive (partition) side
                sc_ps = scp_pool.tile([128, npch], f32, tag="sc")
                for c in range(npch):
                    nc.tensor.matmul(
                        sc_ps[:, c : c + 1],
                        lhsT=h0[:, c * 128 : (c + 1) * 128],
                        rhs=w2_sb[:],
                        start=True, stop=True,
                    )
                # bias = -(s_pos + pospad) per partition chunk
                ppc_sb = spool.tile([128, npch], f32, tag="ppc")
                nc.sync.dma_start(ppc_sb[:], ppcol_d[b, :, :])
                bcol_sb = spool.tile([128, npch], f32, tag="bcol")
                nc.vector.scalar_tensor_tensor(
                    bcol_sb[:], sc_ps[:], -1.0, ppc_sb[:],
                    op0=mybir.AluOpType.mult, op1=mybir.AluOpType.subtract,
                )
                # s_neg row: s + negpad on the free side
                ngr_sb = spool.tile([1, nw], f32, tag="ngr")
                nc.sync.dma_start(ngr_sb[:], negrow_d[b : b + 1, :])
                snr_sb = spool.tile([1, nw], f32, tag="snr")
                nc.vector.tensor_add(snr_sb[:], sr_ps[0:1, nst:K], ngr_sb[:])
                # broadcast s_neg across partitions (f32 matmuls, small)
                pb_ps = pb_pool.tile([128, nw], f32, tag="pb")
                nc.tensor.matmul(
                    pb_ps[:, 0:512], lhsT=ones_sb[:], rhs=snr_sb[0:1, 0:512],
                    start=True, stop=True,
                )
                nc.tensor.matmul(
                    pb_ps[:, 512:nw], lhsT=ones_sb[:], rhs=snr_sb[0:1, 512:nw],
                    start=True, stop=True,
                )
                sbc_sb = spool.tile([128, nw], f32, tag="sbc")
                nc.vector.tensor_copy(sbc_sb[:], pb_ps[:])
                # ---- phase B: softplus(s_neg_i - s_pos_j), accumulate over i ----
                # softplus(x) = ln(exp(x) + 1); one ACT table set for both.
                for c in range(npch):
                    et = tpool.tile([128, nw], f32, tag="tsp")
                    nc.scalar.activation(
                        et[:], sbc_sb[:],
                        mybir.ActivationFunctionType.Exp,
                        bias=bcol_sb[:, c : c + 1], scale=1.0,
                    )
                    tt = tpool.tile([128, nw], f32, tag="tsp2")
                    nc.scalar.activation(
                        tt[:], et[:],
                        mybir.ActivationFunctionType.Ln,
                        bias=1.0, scale=1.0,
                        accum_out=acc_sb[:, b * npch + c : b * npch + c + 1],
                    )

            nc.sync.dma_start(out_d[:], acc_sb[:])

    nc.compile()
    return nc


def kernel(keys, tgts, knn_tgts, mask, W1, b1, W2, b2, _profile=False):
    import ml_dtypes

    from concourse.bass_utils import run_bass_kernel_spmd

    keys = np.asarray(keys, dtype=np.float32)
    tgts = np.asarray(tgts)
    knn_tgts = np.asarray(knn_tgts)
    mask = np.asarray(mask).astype(bool)
    W1 = np.asarray(W1, dtype=np.float32)
    b1 = np.asarray(b1, dtype=np.float32)
    W2 = np.asarray(W2, dtype=np.float32)

    # ---- host-side label/permutation prep ----
    y = knn_tgts == tgts[:, None]
    pos = y & mask
    neg = (~y) & mask
    P = pos.sum(axis=1)
    N_ = neg.sum(axis=1)
    cnt = float((P.astype(np.int64) * N_.astype(np.int64)).sum())

    # stable order: positives, negatives, masked-out
    rank = np.where(pos, 0, np.where(neg, 1, 2)).astype(np.int8)
    order = np.argsort(rank, axis=1, kind="stable")  # [B, K]

    Pmax = int(P.max())
    Pmin = int(P.min())
    assert Pmax <= 512, f"positive count {Pmax} > 512 unsupported"
    Jmax = min(512, ((Pmax + 7) // 8) * 8)
    npch = (Jmax + 127) // 128
    nst = min(Pmin, 512)  # negative free region start (s_row slice origin)
    nw = K - nst

    # permuted, transposed keys: [B, D, K]
    keys_perm = np.take_along_axis(keys, order[:, :, None], axis=1)  # [B, K, D]
    keys_t = np.ascontiguousarray(keys_perm.transpose(0, 2, 1))
    kdt = ml_dtypes.bfloat16 if USE_BF16 else np.float32
    keys_t = keys_t.astype(kdt)

    # pads in permuted coordinates
    kidx = np.arange(K)[None, :]
    pospad = np.where(kidx < P[:, None], 0.0, PAD).astype(np.float32)  # [B, K]
    negpad = np.where(
        (kidx >= P[:, None]) & (kidx < (P + N_)[:, None]), 0.0, -PAD
    ).astype(np.float32)
    # [B, 128, npch] column layout for the positive partition side
    ppcol = np.ascontiguousarray(
        pospad[:, : npch * 128].reshape(B, npch, 128).transpose(0, 2, 1)
    )
    negrow = np.ascontiguousarray(negpad[:, nst:])  # [B, nw]

    w1t = np.ascontiguousarray(W1.T).astype(kdt)  # [D, H]
    w2c = np.ascontiguousarray(W2.reshape(1, H).T).astype(kdt)  # [H, 1]
    b1c = np.ascontiguousarray(b1.reshape(H, 1))

    key = (Jmax, nst, USE_BF16)
    if key not in _cache:
        _cache[key] = _build_program(Jmax, nst, USE_BF16)
    nc = _cache[key]

    in_maps = []
    for c in range(N_CORES):
        sl = slice(c * BPC, (c + 1) * BPC)
        in_maps.append(
            {
                "keys_t": keys_t[sl],
                "w1t": w1t,
                "w2c": w2c,
                "b1c": b1c,
                "ppcol": ppcol[sl],
                "negrow": negrow[sl],
            }
        )

    res = run_bass_kernel_spmd(
        nc, in_maps, list(range(N_CORES)), trace=bool(_profile)
    )
    total = 0.0
    for r in res.results:
        total += float(r["acc_out"].astype(np.float64).sum())
    if _profile:
        print(f"HW exec time: {res.exec_time_ns} ns")
        globals()["_last_results"] = res
    loss = np.float64(total) / np.float64(cnt)
    return np.array(loss, dtype=np.float32)
